# revision 8
# baseline (speedup 1.0000x reference)
"""Trainium2 Bass kernel for nn_CliffordEPBottleneckV2.

Math:
    h0 = x @ W_in + b_in                      (B, HID) viewed as (B, OUT, 8)
    EP:  h <- h - 0.01*(h + 0.1*h@(We+We.T))  x3   (linear! h3 = h0 @ M^3 on blade axis)
    out = h3_flat @ W_out + b_out

Each EP step is linear in h, so the whole relaxation is one 8x8 matrix
M3 = (0.99*I - 0.001*(We+We.T))^3 applied on the blade axis.  We fold M3
into W_out rows on the host (cheap: 8x8 @ reshaped W_out) and fold b_in
through as well:

    out = x @ W_in @ W_out_eff + (b_in @ W_out_eff + b_out)

Device computation (the 275 GFLOP that matters):
    partial_c = x @ W_in[:, c] @ W_out_eff[c, :]        per core c

Sharding: tensor-parallel over the HID=16384 dimension across 8 cores
(2048 hid per core).  Unsharding sums the 8 partials and adds the folded
bias (a 16.8 MFLOP reduction = 0.006% of the device FLOPs; the on-device
collective stack in this environment costs 2x the whole computation, so
the reduction belongs in the host gather step).

Per-core kernel: two fused matmul stages in fp32r (FP22 mantissa-
truncated fp32, full PE rate at free-dim>=256).  x^T (16MB) stays
resident in SBUF; hid is processed in two phases of 1024 so the h^T
phase buffer (4MB) fits beside it; weights stream from HBM exactly
once.  Phase 0 writes the output, phase 1 accumulates into it with a
CCE add in the DMA path.
"""

import numpy as np

B, IN_DIM, OUT_DIM = 1024, 4096, 2048
N_BLADES = 8
HID = OUT_DIM * N_BLADES
N_CORES = 8
HPC = HID // N_CORES          # 2048 hid per core
JT = HPC // 128               # 16 j-tiles per core
PHASES = 2
JP = JT // PHASES             # 8 j-tiles per phase
KO = IN_DIM // 128            # 32 contraction tiles, stage 1
BSLAB = B // 512              # 2 moving slabs, stage 1
N2 = OUT_DIM // 512           # 4 out-column slabs
BSUB = B // 128               # 8 batch subtiles, stage 2

_CACHE = {}


def _build_bass(reps=1):
    # reps>1 chains the whole computation end-to-end `reps` times inside one
    # NEFF; test harnesses use the time-vs-reps slope to measure the
    # steady-state kernel time underneath the multi-ms dispatch overhead of
    # this environment.  kernel() always uses reps=1.
    import concourse.bacc as bacc
    import concourse.mybir as mybir
    import concourse.tile as tile

    f32 = mybir.dt.float32
    f32r = mybir.dt.float32r

    nc = bacc.Bacc(
        "TRN2", target_bir_lowering=False, debug=False, num_devices=N_CORES
    )

    xt_d = nc.dram_tensor("xt", [128, KO, B], f32r, kind="ExternalInput").ap()
    win_d = nc.dram_tensor("win", [128, JT, KO, 128], f32r, kind="ExternalInput").ap()
    wout_d = nc.dram_tensor("wout", [128, JT, OUT_DIM], f32r, kind="ExternalInput").ap()
    out_d = nc.dram_tensor("out", [B, OUT_DIM], f32, kind="ExternalOutput").ap()

    with tile.TileContext(nc) as tc:
        with (
            tc.tile_pool(name="xpool", bufs=1) as xpool,
            tc.tile_pool(name="hpool", bufs=1) as hpool,
            tc.tile_pool(name="wpool", bufs=3) as wpool,
            tc.tile_pool(name="vpool", bufs=8) as vpool,
            tc.tile_pool(name="spool", bufs=3) as spool,
            tc.tile_pool(name="ps1", bufs=2, space="PSUM") as ps1,
            tc.tile_pool(name="ps2", bufs=3, space="PSUM") as ps2,
        ):
            xt_sb = xpool.tile([128, KO, B], f32r)
            for ko in range(KO):
                nc.sync.dma_start(xt_sb[:, ko, :], xt_d[:, ko, :])

            for _rep, ph in [(r, p) for r in range(reps) for p in range(PHASES)]:
                hT = hpool.tile([128, JP, B], f32r, name="hT")
                # ---- stage 1: hT[:, jl, :] = (x @ W_in[:, jslice]).T
                for jl in range(JP):
                    jt = ph * JP + jl
                    chunks = []
                    for q in range(2):
                        wc = wpool.tile([128, KO // 2, 128], f32r, name="winc")
                        nc.sync.dma_start(
                            wc[:], win_d[:, jt, q * (KO // 2):(q + 1) * (KO // 2), :]
                        )
                        chunks.append(wc)
                    for bs in range(BSLAB):
                        pt = ps1.tile([128, 512], f32, name="ps1t")
                        for ko in range(KO):
                            nc.tensor.matmul(
                                pt[:],
                                chunks[ko // (KO // 2)][:, ko % (KO // 2), :],
                                xt_sb[:, ko, bs * 512:(bs + 1) * 512],
                                start=(ko == 0),
                                stop=(ko == KO - 1),
                            )
                        nc.vector.tensor_copy(
                            hT[:, jl, bs * 512:(bs + 1) * 512], pt[:]
                        )
                # ---- stage 2: out += hT.T @ W_out_eff[jslice, :]
                for n2 in range(N2):
                    wts = []
                    for jl in range(JP):
                        jt = ph * JP + jl
                        wt = vpool.tile([128, 512], f32r, name="woutt")
                        nc.sync.dma_start(
                            wt[:], wout_d[:, jt, n2 * 512:(n2 + 1) * 512]
                        )
                        wts.append(wt)
                    for bu in range(BSUB):
                        pt2 = ps2.tile([128, 512], f32, name="ps2t")
                        for jl in range(JP):
                            nc.tensor.matmul(
                                pt2[:],
                                hT[:, jl, bu * 128:(bu + 1) * 128],
                                wts[jl][:],
                                start=(jl == 0),
                                stop=(jl == JP - 1),
                            )
                        ot = spool.tile([128, 512], f32, name="outt")
                        nc.vector.tensor_copy(ot[:], pt2[:])
                        dst = out_d[bu * 128:(bu + 1) * 128, n2 * 512:(n2 + 1) * 512]
                        if ph == 0:
                            nc.sync.dma_start(dst, ot[:])
                        else:
                            nc.gpsimd.dma_start(
                                dst, ot[:], accum_op=mybir.AluOpType.add
                            )

    nc.compile()
    return nc


def get_nc(reps=1):
    key = ("nc", reps)
    if key not in _CACHE:
        _CACHE[key] = _build_bass(reps)
    return _CACHE[key]


def fold_weights(W_in, b_in, W_e, W_out, b_out):
    """EP fold on the host: returns (W_out_eff, bias_total)."""
    W_sym = (W_e + W_e.T).astype(np.float64)
    M = 0.99 * np.eye(N_BLADES) - 0.001 * W_sym
    M3 = (M @ M @ M).astype(np.float32)
    Wr = np.asarray(W_out, np.float32).reshape(OUT_DIM, N_BLADES, OUT_DIM)
    W_out_eff = np.tensordot(M3, Wr, axes=(1, 1)).transpose(1, 0, 2).reshape(HID, OUT_DIM)
    W_out_eff = np.ascontiguousarray(W_out_eff)
    bias_total = np.asarray(b_in, np.float32) @ W_out_eff + np.asarray(b_out, np.float32)
    return W_out_eff, bias_total


def prepare_in_maps(x, W_in, b_in, W_e, W_out, b_out):
    """Host-side fold + shard: returns (per-core input maps, bias_total)."""
    x = np.ascontiguousarray(np.asarray(x, dtype=np.float32))
    W_in = np.ascontiguousarray(np.asarray(W_in, dtype=np.float32))
    W_out_eff, bias_total = fold_weights(W_in, b_in, W_e, W_out, b_out)

    # x^T tiled: xt[p, ko, b] = x[b, ko*128+p]
    xt = np.ascontiguousarray(x.reshape(B, KO, 128).transpose(2, 1, 0))

    in_maps = []
    for c in range(N_CORES):
        Wc = W_in[:, c * HPC:(c + 1) * HPC]                    # (IN_DIM, HPC)
        win = np.ascontiguousarray(
            Wc.reshape(KO, 128, JT, 128).transpose(1, 2, 0, 3)
        )                                                      # (128, JT, KO, 128)
        We_c = W_out_eff[c * HPC:(c + 1) * HPC, :]             # (HPC, OUT_DIM)
        wout = np.ascontiguousarray(
            We_c.reshape(JT, 128, OUT_DIM).transpose(1, 0, 2)
        )                                                      # (128, JT, OUT_DIM)
        in_maps.append({"xt": xt, "win": win, "wout": wout})
    return in_maps, bias_total


def assemble(results, bias_total):
    """Unshard the tensor-parallel partials: sum over cores, add bias."""
    acc = results[0]["out"].astype(np.float32).copy()
    for c in range(1, N_CORES):
        acc += results[c]["out"]
    acc += bias_total[None, :]
    return acc


def kernel(x, W_in, b_in, W_e, W_out, b_out):
    from concourse.bass_utils import run_bass_kernel_spmd

    nc = get_nc()
    in_maps, bias_total = prepare_in_maps(x, W_in, b_in, W_e, W_out, b_out)
    res = run_bass_kernel_spmd(nc, in_maps, core_ids=list(range(N_CORES)))
    return assemble(res.results, bias_total)


# revision 10
# speedup vs baseline: 2.9132x; 2.9132x over previous
"""Trainium2 Bass kernel for nn_CliffordEPBottleneckV2.

Math:
    h0 = x @ W_in + b_in                      (B, HID) viewed as (B, OUT, 8)
    EP:  h <- h - 0.01*(h + 0.1*h@(We+We.T))  x3   (linear! h3 = h0 @ M^3 on blade axis)
    out = h3_flat @ W_out + b_out

Each EP step is linear in h, so the whole relaxation is one 8x8 matrix
M3 = (0.99*I - 0.001*(We+We.T))^3 applied on the blade axis.  We fold M3
into W_out rows on the host (cheap: 8x8 @ reshaped W_out) and fold b_in
through as well:

    out = x @ W_in @ W_out_eff + (b_in @ W_out_eff + b_out)

Device computation (the 275 GFLOP that matters):
    partial_c = x @ W_in[:, c] @ W_out_eff[c, :]        per core c

Sharding: tensor-parallel over the HID=16384 dimension across 8 cores
(2048 hid per core).  Unsharding sums the 8 partials and adds the folded
bias (a 16.8 MFLOP reduction = 0.006% of the device FLOPs; the on-device
collective stack in this environment costs 2x the whole computation, so
the reduction belongs in the host gather step).

Per-core kernel: two fused matmul stages in fp32r (FP22 mantissa-
truncated fp32, full PE rate at free-dim>=256).  x^T (16MB) stays
resident in SBUF; hid is processed in two phases of 1024 so the h^T
phase buffer (4MB) fits beside it; weights stream from HBM exactly
once.  Each phase writes its own output tensor via HWDGE (the gpsimd
SWDGE CCE-accumulate path measured ~630us/call slower); the host sums
the 2x8 partials in the unshard step.
"""

import numpy as np

B, IN_DIM, OUT_DIM = 1024, 4096, 2048
N_BLADES = 8
HID = OUT_DIM * N_BLADES
N_CORES = 8
HPC = HID // N_CORES          # 2048 hid per core
JT = HPC // 128               # 16 j-tiles per core
PHASES = 2
JP = JT // PHASES             # 8 j-tiles per phase
KO = IN_DIM // 128            # 32 contraction tiles, stage 1
BSLAB = B // 512              # 2 moving slabs, stage 1
N2 = OUT_DIM // 512           # 4 out-column slabs
BSUB = B // 128               # 8 batch subtiles, stage 2
BPC = B // N_CORES            # batch rows per core (used by bench scripts)

_CACHE = {}


def _build_bass(reps=1):
    # reps>1 chains the whole computation end-to-end `reps` times inside one
    # NEFF; test harnesses use the time-vs-reps slope to measure the
    # steady-state kernel time underneath the multi-ms dispatch overhead of
    # this environment.  kernel() always uses reps=1.
    import concourse.bacc as bacc
    import concourse.mybir as mybir
    import concourse.tile as tile

    f32 = mybir.dt.float32
    f32r = mybir.dt.float32r

    nc = bacc.Bacc(
        "TRN2", target_bir_lowering=False, debug=False, num_devices=N_CORES
    )

    xt_d = nc.dram_tensor("xt", [128, KO, B], f32r, kind="ExternalInput").ap()
    win_d = nc.dram_tensor("win", [128, JT, KO, 128], f32r, kind="ExternalInput").ap()
    wout_d = nc.dram_tensor("wout", [128, JT, OUT_DIM], f32r, kind="ExternalInput").ap()
    outs_d = [
        nc.dram_tensor(f"out{p}", [B, OUT_DIM], f32, kind="ExternalOutput").ap()
        for p in range(PHASES)
    ]

    with tile.TileContext(nc) as tc:
        with (
            tc.tile_pool(name="xpool", bufs=1) as xpool,
            tc.tile_pool(name="hpool", bufs=1) as hpool,
            tc.tile_pool(name="wpool", bufs=3) as wpool,
            tc.tile_pool(name="vpool", bufs=8) as vpool,
            tc.tile_pool(name="spool", bufs=3) as spool,
            tc.tile_pool(name="ps1", bufs=2, space="PSUM") as ps1,
            tc.tile_pool(name="ps2", bufs=3, space="PSUM") as ps2,
        ):
            xt_sb = xpool.tile([128, KO, B], f32r)
            for ko in range(KO):
                nc.sync.dma_start(xt_sb[:, ko, :], xt_d[:, ko, :])

            for _rep, ph in [(r, p) for r in range(reps) for p in range(PHASES)]:
                hT = hpool.tile([128, JP, B], f32r, name="hT")
                # ---- stage 1: hT[:, jl, :] = (x @ W_in[:, jslice]).T
                for jl in range(JP):
                    jt = ph * JP + jl
                    chunks = []
                    for q in range(2):
                        wc = wpool.tile([128, KO // 2, 128], f32r, name="winc")
                        nc.sync.dma_start(
                            wc[:], win_d[:, jt, q * (KO // 2):(q + 1) * (KO // 2), :]
                        )
                        chunks.append(wc)
                    for bs in range(BSLAB):
                        pt = ps1.tile([128, 512], f32, name="ps1t")
                        for ko in range(KO):
                            nc.tensor.matmul(
                                pt[:],
                                chunks[ko // (KO // 2)][:, ko % (KO // 2), :],
                                xt_sb[:, ko, bs * 512:(bs + 1) * 512],
                                start=(ko == 0),
                                stop=(ko == KO - 1),
                            )
                        nc.vector.tensor_copy(
                            hT[:, jl, bs * 512:(bs + 1) * 512], pt[:]
                        )
                # ---- stage 2: out += hT.T @ W_out_eff[jslice, :]
                for n2 in range(N2):
                    wts = []
                    for jl in range(JP):
                        jt = ph * JP + jl
                        wt = vpool.tile([128, 512], f32r, name="woutt")
                        nc.sync.dma_start(
                            wt[:], wout_d[:, jt, n2 * 512:(n2 + 1) * 512]
                        )
                        wts.append(wt)
                    for bu in range(BSUB):
                        pt2 = ps2.tile([128, 512], f32, name="ps2t")
                        for jl in range(JP):
                            nc.tensor.matmul(
                                pt2[:],
                                hT[:, jl, bu * 128:(bu + 1) * 128],
                                wts[jl][:],
                                start=(jl == 0),
                                stop=(jl == JP - 1),
                            )
                        ot = spool.tile([128, 512], f32, name="outt")
                        nc.vector.tensor_copy(ot[:], pt2[:])
                        dst = outs_d[ph][bu * 128:(bu + 1) * 128,
                                         n2 * 512:(n2 + 1) * 512]
                        nc.sync.dma_start(dst, ot[:])

    nc.compile()
    return nc


def get_nc(reps=1):
    key = ("nc", reps)
    if key not in _CACHE:
        _CACHE[key] = _build_bass(reps)
    return _CACHE[key]


def fold_weights(W_in, b_in, W_e, W_out, b_out):
    """EP fold on the host: returns (W_out_eff, bias_total)."""
    W_sym = (W_e + W_e.T).astype(np.float64)
    M = 0.99 * np.eye(N_BLADES) - 0.001 * W_sym
    M3 = (M @ M @ M).astype(np.float32)
    Wr = np.asarray(W_out, np.float32).reshape(OUT_DIM, N_BLADES, OUT_DIM)
    W_out_eff = np.tensordot(M3, Wr, axes=(1, 1)).transpose(1, 0, 2).reshape(HID, OUT_DIM)
    W_out_eff = np.ascontiguousarray(W_out_eff)
    bias_total = np.asarray(b_in, np.float32) @ W_out_eff + np.asarray(b_out, np.float32)
    return W_out_eff, bias_total


def prepare_in_maps(x, W_in, b_in, W_e, W_out, b_out):
    """Host-side fold + shard: returns (per-core input maps, bias_total)."""
    x = np.ascontiguousarray(np.asarray(x, dtype=np.float32))
    W_in = np.ascontiguousarray(np.asarray(W_in, dtype=np.float32))
    W_out_eff, bias_total = fold_weights(W_in, b_in, W_e, W_out, b_out)

    # x^T tiled: xt[p, ko, b] = x[b, ko*128+p]
    xt = np.ascontiguousarray(x.reshape(B, KO, 128).transpose(2, 1, 0))

    in_maps = []
    for c in range(N_CORES):
        Wc = W_in[:, c * HPC:(c + 1) * HPC]                    # (IN_DIM, HPC)
        win = np.ascontiguousarray(
            Wc.reshape(KO, 128, JT, 128).transpose(1, 2, 0, 3)
        )                                                      # (128, JT, KO, 128)
        We_c = W_out_eff[c * HPC:(c + 1) * HPC, :]             # (HPC, OUT_DIM)
        wout = np.ascontiguousarray(
            We_c.reshape(JT, 128, OUT_DIM).transpose(1, 0, 2)
        )                                                      # (128, JT, OUT_DIM)
        in_maps.append({"xt": xt, "win": win, "wout": wout})
    return in_maps, bias_total


def assemble(results, bias_total):
    """Unshard the tensor-parallel partials: sum over cores and hid-phases,
    add the folded bias."""
    acc = results[0]["out0"].astype(np.float32).copy()
    acc += results[0]["out1"]
    for c in range(1, N_CORES):
        acc += results[c]["out0"]
        acc += results[c]["out1"]
    acc += bias_total[None, :]
    return acc


def kernel(x, W_in, b_in, W_e, W_out, b_out):
    from concourse.bass_utils import run_bass_kernel_spmd

    nc = get_nc()
    in_maps, bias_total = prepare_in_maps(x, W_in, b_in, W_e, W_out, b_out)
    res = run_bass_kernel_spmd(nc, in_maps, core_ids=list(range(N_CORES)))
    return assemble(res.results, bias_total)
